# revision 1
# baseline (speedup 1.0000x reference)
"""GQA attention + RoPE + causal softmax + output projection on 8 TRN2 cores.

Sharding: tensor-parallel over heads. Core i owns q-heads [4i, 4i+4) and
kv-head i (GQA group size 4 aligns exactly with HQ/8=4, HK/8=1).

Per-core pipeline (everything in transposed "feature-on-partitions" layout):
  A) Q^T/K^T/V^T projections:  lhsT = weight chunk [Dc,128], rhs = x^T chunk
     [Dc, 512]  ->  PSUM [feat, seq].  RoPE applied on Q^T/K^T via
     stream_shuffle (pair swap across partitions) + 2 muls + add.
     V^T is PE-transposed into V [seq, 128] for the PV matmul.
  B) Attention per (head, 512-wide q-block), causal, SW-pipelined by 2
     chunks so the PE never waits on the ACT exp latency:
     scores^T [sk,128 x sq,512] = K^T-chunk (stationary) x Q^T (moving);
     p = exp(scores * 1/sqrt(hd)) on ACT (no max subtraction - scores are
     O(1e-2) for this problem); diagonal tiles masked with a 0/1 mask (DVE);
     norm via all-ones [128,128] stationary matmul accumulating the column
     sums - every output partition gets the softmax denominator, i.e. the
     broadcast comes for free;  out^T[128,sq] += V-chunk^T @ p (PE);
     final: attn^T = out^T * reciprocal_approx_fast(norm)  (DVE).
  C) Per-head AllGather of attn^T [128, S] (bf16) -> [1024, S], issued as
     each head finishes so the collectives overlap later heads' compute.
  D) out^T column-shard:  lhsT = wo chunk, rhs = gathered attn^T chunk,
     accumulated over all 4096 contraction rows.  Host transposes + concats.

Matmul operands are bf16 (1 cycle/row on PE); accumulation is fp32 in PSUM;
softmax denominator and normalization stay fp32.
"""

import numpy as np
import ml_dtypes

import concourse.bass as bass
import concourse.mybir as mybir
import concourse.tile as tile
from concourse import bacc
from concourse.bass_utils import run_bass_kernel_spmd

# Problem dims (hardcoded per contract)
B, S, D = 1, 2048, 4096
HQ, HK, HD = 32, 8, 128
NCORES = 8
HQL = HQ // NCORES          # 4 local q heads
SB = 512                    # seq block (matmul moving free dim)
NB = S // SB                # 4 seq blocks
NC_ = D // 128              # 32 contraction chunks for D
NKC = S // 128              # 16 sk chunks
SCALE = 1.0 / float(np.sqrt(HD))

F32 = mybir.dt.float32
F32R = mybir.dt.float32r
BF16 = mybir.dt.bfloat16

# stream_shuffle mask: swap adjacent pairs within each 32-partition quadrant
SWAP_MASK = [(i ^ 1) for i in range(32)]


def _build_nc():
    nc = bacc.Bacc(
        "TRN2", target_bir_lowering=False, debug=False, num_devices=NCORES
    )

    io = {}
    io["xT"] = nc.dram_tensor("xT", [D, S], BF16, kind="ExternalInput")
    io["wq"] = nc.dram_tensor("wq", [D, HQL * HD], BF16, kind="ExternalInput")
    io["wk"] = nc.dram_tensor("wk", [D, HD], BF16, kind="ExternalInput")
    io["wv"] = nc.dram_tensor("wv", [D, HD], BF16, kind="ExternalInput")
    io["wo"] = nc.dram_tensor("wo", [D, HQL * HD], BF16, kind="ExternalInput")
    io["cos2"] = nc.dram_tensor("cos2", [HD, S], BF16, kind="ExternalInput")
    io["sin2"] = nc.dram_tensor("sin2", [HD, S], BF16, kind="ExternalInput")
    io["maskt"] = nc.dram_tensor("maskt", [128, NB, SB], BF16, kind="ExternalInput")
    io["ident"] = nc.dram_tensor("ident", [128, 128], BF16, kind="ExternalInput")
    io["outT"] = nc.dram_tensor("outT", [HQL * HD, S], F32, kind="ExternalOutput")

    with tile.TileContext(nc) as tc:
        _body(tc, io)
    nc.compile()
    return nc


def _body(tc, io):
    nc = tc.nc
    from contextlib import ExitStack

    ctx = ExitStack()
    with ctx:
        consts = ctx.enter_context(tc.tile_pool(name="consts", bufs=1))
        qkv = ctx.enter_context(tc.tile_pool(name="qkv", bufs=1))
        dram = ctx.enter_context(tc.tile_pool(name="dram", bufs=1, space="DRAM"))

        # ---- constants (DMAs emitted inside the s-block loop, trailing that
        # block's xt loads on the scalar queue, so startup isn't delayed) ----
        cos2 = consts.tile([HD, S], BF16)
        sin2 = consts.tile([HD, S], BF16)
        ident = consts.tile([128, 128], BF16)
        maskt = consts.tile([128, NB, SB], BF16)
        ones_mat = consts.tile([128, 128], BF16)
        nc.vector.memset(ones_mat, 1.0)

        # ---- persistent per-core tensors (split per s-block so stage B's
        # dependencies are fine-grained: block (h,qb) only waits for the
        # s-blocks it actually reads) ----
        qt_sb = [
            qkv.tile([HD, HQL, SB], BF16, name=f"qt{sb}") for sb in range(NB)
        ]
        kt_sb = [qkv.tile([HD, SB], BF16, name=f"kt{sb}") for sb in range(NB)]
        vs_sb = [
            qkv.tile([128, SB // 128, HD], BF16, name=f"vs{sb}") for sb in range(NB)
        ]

        # per-head bounce + gather buffers (chunked collective -> overlap)
        attn_loc = [
            dram.tile([HD, S], BF16, name=f"attn_loc{h}") for h in range(HQL)
        ]
        attn_g = [
            dram.tile([NCORES * HD, S], BF16, name=f"attn_g{h}", addr_space="Shared")
            for h in range(HQL)
        ]

        # ================= Stage A: projections + RoPE =================
        with ctx_pools(tc) as (wpool, xpool, rpool, psA):
            wq_sb = wpool.tile([128, NC_, HQL * HD], BF16)
            wk_sb = wpool.tile([128, NC_, HD], BF16)
            wv_sb = wpool.tile([128, NC_, HD], BF16)
            # first chunks as singles (c=0 even finer) so the PE starts ASAP
            for t in range(HQL):
                nc.gpsimd.dma_start(
                    out=wq_sb[:, 0, t * 128 : (t + 1) * 128],
                    in_=io["wq"][0:128, t * 128 : (t + 1) * 128],
                )
            nc.gpsimd.dma_start(out=wk_sb[:, 0, :], in_=io["wk"][0:128, :])
            nc.gpsimd.dma_start(out=wv_sb[:, 0, :], in_=io["wv"][0:128, :])
            for c in range(1, 4):
                sl = slice(c * 128, (c + 1) * 128)
                nc.gpsimd.dma_start(out=wq_sb[:, c, :], in_=io["wq"][sl, :])
                nc.gpsimd.dma_start(out=wk_sb[:, c, :], in_=io["wk"][sl, :])
                nc.gpsimd.dma_start(out=wv_sb[:, c, :], in_=io["wv"][sl, :])
            for c4 in range(1, NC_ // 4):
                sl = slice(c4 * 4 * 128, (c4 * 4 + 4) * 128)
                nc.gpsimd.dma_start(
                    out=wq_sb[:, c4 * 4 : c4 * 4 + 4, :],
                    in_=io["wq"][sl, :].rearrange("(c p) n -> p c n", p=128),
                )
                nc.gpsimd.dma_start(
                    out=wk_sb[:, c4 * 4 : c4 * 4 + 4, :],
                    in_=io["wk"][sl, :].rearrange("(c p) n -> p c n", p=128),
                )
                nc.gpsimd.dma_start(
                    out=wv_sb[:, c4 * 4 : c4 * 4 + 4, :],
                    in_=io["wv"][sl, :].rearrange("(c p) n -> p c n", p=128),
                )

            for sb in range(NB):
                ssl = slice(sb * SB, (sb + 1) * SB)
                ps_q = [
                    psA.tile(
                        [128, SB],
                        F32,
                        name=f"psq{t}_{sb}",
                        tag=f"psq{t}",
                        # double-buffer the first tag: the next s-block's
                        # opening matmul then never waits on this block's
                        # ACT eviction (7 banks total + this extra = 8)
                        bufs=2 if t == 0 else 1,
                    )
                    for t in range(HQL)
                ]
                ps_k = psA.tile([128, SB], F32, tag="psk")
                ps_v = psA.tile([128, SB], F32, tag="psv")
                for c in range(NC_):
                    xt = xpool.tile([128, SB], BF16, tag="xt")
                    xt_eng = nc.sync if c % 2 == 0 else nc.scalar
                    xt_eng.dma_start(
                        out=xt, in_=io["xT"][c * 128 : (c + 1) * 128, ssl]
                    )
                    first, last = c == 0, c == NC_ - 1
                    for t in range(HQL):
                        nc.tensor.matmul(
                            ps_q[t],
                            lhsT=wq_sb[:, c, t * 128 : (t + 1) * 128],
                            rhs=xt,
                            start=first,
                            stop=last,
                        )
                    nc.tensor.matmul(
                        ps_k, lhsT=wk_sb[:, c, :], rhs=xt, start=first, stop=last
                    )
                    nc.tensor.matmul(
                        ps_v, lhsT=wv_sb[:, c, :], rhs=xt, start=first, stop=last
                    )

                # const loads trail this block's xt DMAs on the scalar queue
                if sb == 0:
                    nc.scalar.dma_start(out=ident, in_=io["ident"][:, :])
                nc.scalar.dma_start(out=cos2[:, ssl], in_=io["cos2"][:, ssl])
                nc.scalar.dma_start(out=sin2[:, ssl], in_=io["sin2"][:, ssl])
                if sb == 1:
                    nc.scalar.dma_start(out=maskt, in_=io["maskt"][:, :, :])

                # V^T -> V first (PE transpose per 128-col chunk): its DVE
                # copies release the psvt bank before the rope chains queue up
                vts = rpool.tile([128, SB], BF16, name=f"vts{sb}", tag="vts")
                nc.scalar.copy(vts, ps_v)
                for u in range(SB // 128):
                    ps_vt = psA.tile([128, 128], BF16, name=f"psvt{sb}_{u}", tag="psvt")
                    nc.tensor.transpose(
                        ps_vt, vts[:, u * 128 : (u + 1) * 128], ident
                    )
                    nc.vector.tensor_copy(vs_sb[sb][:, u, :], ps_vt)

                # RoPE: rot(q) = q * cos2 + pairswap(q) * sin2   (sign in sin2).
                # ACT-copy the psum out first (casting bf16): frees the PSUM
                # bank immediately and halves the DVE chain cost.
                def rope(ps, dst, idx):
                    qc = rpool.tile([128, SB], BF16, name=f"qc{idx}", tag="qc")
                    nc.scalar.copy(qc, ps)
                    sw = rpool.tile([128, SB], BF16, name=f"sw{idx}", tag="sw")
                    nc.vector.stream_shuffle(sw, qc, SWAP_MASK)
                    t1 = rpool.tile([128, SB], BF16, name=f"t1{idx}", tag="t1")
                    nc.vector.tensor_mul(t1, qc, cos2[:, ssl])
                    t2 = rpool.tile([128, SB], BF16, name=f"t2{idx}", tag="t2")
                    nc.vector.tensor_mul(t2, sw, sin2[:, ssl])
                    nc.vector.tensor_add(dst, t1, t2)

                for t in range(HQL):
                    rope(ps_q[t], qt_sb[sb][:, t, :], f"q{sb}_{t}")
                rope(ps_k, kt_sb[sb], f"k{sb}")

        # wo loads placed here so they fill DMA idle time during stage B
        wo_pool = ctx.enter_context(tc.tile_pool(name="wo_pool", bufs=1))
        wo_sb = wo_pool.tile([128, NC_, HQL * HD], BF16)
        for c4 in range(NC_ // 4):
            sl = slice(c4 * 4 * 128, (c4 * 4 + 4) * 128)
            nc.gpsimd.dma_start(
                out=wo_sb[:, c4 * 4 : c4 * 4 + 4, :],
                in_=io["wo"][sl, :].rearrange("(c p) n -> p c n", p=128),
            )

        # stage-D SBUF pools opened before stage B: the at-tile prefetch can
        # then run into addresses disjoint from stage-B pools.
        apool = ctx.enter_context(tc.tile_pool(name="apool", bufs=8))
        opool = ctx.enter_context(tc.tile_pool(name="opool", bufs=4))

        # ================= Stage B: attention =================
        # Software-pipelined by one chunk: norm/pv of chunk kc-1 are emitted
        # after scores/exp of chunk kc, so the PE never sits on the ACT exp
        # latency at block boundaries.
        with ctx_pools_b(tc) as (ppool, spool, psB):
            for h in range(HQL):
                for qb in range(NB):
                    qsl = slice(qb * SB, (qb + 1) * SB)
                    nkc = (qb + 1) * (SB // 128)
                    ps_o = psB.tile([128, SB], F32, name=f"pso{h}_{qb}", tag="pso")
                    # ones_mat as stationary => every partition of ps_n gets
                    # the column-sum: the softmax denom, already broadcast.
                    ps_n = psB.tile([128, SB], F32, name=f"psn{h}_{qb}", tag="psn")
                    pts = {}

                    def consume(kc, h=h, qb=qb, ps_n=ps_n, ps_o=ps_o, pts=pts):
                        first, last = kc == 0, kc == nkc - 1
                        pt = pts.pop(kc)
                        nc.tensor.matmul(
                            ps_n, lhsT=ones_mat, rhs=pt, start=first, stop=last
                        )
                        nc.tensor.matmul(
                            ps_o,
                            lhsT=vs_sb[kc // 4][:, kc % 4, :],
                            rhs=pt,
                            start=first,
                            stop=last,
                        )

                    for kc in range(nkc):
                        ps_s = psB.tile(
                            [128, SB], F32, name=f"pss{h}_{qb}_{kc}", tag="pss", bufs=4
                        )
                        nc.tensor.matmul(
                            ps_s,
                            lhsT=kt_sb[kc // 4][:, (kc % 4) * 128 : (kc % 4 + 1) * 128],
                            rhs=qt_sb[qb][:, h, :],
                            start=True,
                            stop=True,
                        )
                        pt = ppool.tile(
                            [128, SB], BF16, name=f"pt{h}_{qb}_{kc}", tag="pt"
                        )
                        nc.scalar.activation(
                            pt, ps_s, mybir.ActivationFunctionType.Exp, scale=SCALE
                        )
                        td = kc - qb * 4
                        if td >= 0:
                            ptm = ppool.tile(
                                [128, SB], BF16, name=f"ptm{h}_{qb}_{kc}", tag="ptm",
                                bufs=4,
                            )
                            nc.vector.tensor_mul(ptm, pt, maskt[:, td, :])
                            pt = ptm
                        pts[kc] = pt
                        if kc >= 2:
                            consume(kc - 2)
                    if nkc >= 2:
                        consume(nkc - 2)
                    consume(nkc - 1)
                    rb = spool.tile([128, SB], F32, name=f"rb{h}_{qb}", tag="rb")
                    nc.vector.reciprocal_approx_fast(rb, ps_n)
                    # ao bufs=4: ao DMAs sit behind gather triggers on gpsimd,
                    # so a straggler-delayed gather must not back up into DVE
                    ao = spool.tile(
                        [128, SB], BF16, name=f"ao{h}_{qb}", tag="ao", bufs=4
                    )
                    nc.vector.tensor_mul(ao, ps_o, rb)
                    # ao on gpsimd (precedes its trigger there); sync stays
                    # empty after stage A so stage-D loads can pace the gathers
                    nc.gpsimd.dma_start(out=attn_loc[h][:, qsl], in_=ao)
                # gather this head while later heads still compute
                nc.gpsimd.collective_compute(
                    "AllGather",
                    mybir.AluOpType.bypass,
                    replica_groups=[list(range(NCORES))],
                    ins=[attn_loc[h].opt()],
                    outs=[attn_g[h].opt()],
                )

        # ================= Stage D: out = attn @ wo (column shard) =========
        # consume gathered chunks in arrival order: gather j holds rows of
        # q-head j for every core; global wo row chunk = core*4 + j.
        with tc.tile_pool(name="psD", bufs=2, space="PSUM") as psD:
            for g in range(NB):
                gsl = slice(g * SB, (g + 1) * SB)
                ps_d = [
                    psD.tile([128, SB], F32, name=f"psd{g}_{n}", tag=f"psd{n}")
                    for n in range(HQL)
                ]
                for j in range(HQL):
                    for i in range(NCORES):
                        c = i * HQL + j
                        at = apool.tile([128, SB], BF16, tag="at")
                        nc.sync.dma_start(
                            out=at, in_=attn_g[j][i * 128 : (i + 1) * 128, gsl]
                        )
                        first = j == 0 and i == 0
                        last = j == HQL - 1 and i == NCORES - 1
                        for n in range(HQL):
                            nc.tensor.matmul(
                                ps_d[n],
                                lhsT=wo_sb[:, c, n * 128 : (n + 1) * 128],
                                rhs=at,
                                start=first,
                                stop=last,
                            )
                for n in range(HQL):
                    ot = opool.tile([128, SB], F32, name=f"ot{g}_{n}", tag="ot")
                    nc.scalar.copy(ot, ps_d[n])
                    nc.sync.dma_start(
                        out=io["outT"][n * 128 : (n + 1) * 128, gsl], in_=ot
                    )


from contextlib import contextmanager


@contextmanager
def ctx_pools(tc):
    with (
        tc.tile_pool(name="wpool", bufs=1) as wpool,
        tc.tile_pool(name="xpool", bufs=12) as xpool,
        tc.tile_pool(name="rpool", bufs=3) as rpool,
        tc.tile_pool(name="psA", bufs=1, space="PSUM") as psA,
    ):
        yield wpool, xpool, rpool, psA


@contextmanager
def ctx_pools_b(tc):
    with (
        tc.tile_pool(name="ppool", bufs=8) as ppool,
        tc.tile_pool(name="spool", bufs=2) as spool,
        tc.tile_pool(name="psB", bufs=2, space="PSUM") as psB,
    ):
        yield ppool, spool, psB


_NC_CACHE = None


def _get_nc():
    global _NC_CACHE
    if _NC_CACHE is None:
        _NC_CACHE = _build_nc()
    return _NC_CACHE


def _prep_in_maps(x, freqs_cos, freqs_sin, wq, wk, wv, wo):
    bf = ml_dtypes.bfloat16
    x = np.asarray(x, np.float32).reshape(S, D)
    xT = np.ascontiguousarray(x.T).astype(bf)
    cos = np.asarray(freqs_cos, np.float32)  # [S, HD/2]
    sin = np.asarray(freqs_sin, np.float32)
    cos2 = np.repeat(cos.T, 2, axis=0)  # [HD, S], rows 2j,2j+1 = cos[:, j]
    sin_t = sin.T
    sin2 = np.empty((HD, S), np.float32)
    sin2[0::2] = -sin_t
    sin2[1::2] = sin_t
    p = np.arange(128)[:, None, None]
    t = np.arange(NB)[None, :, None]
    c = np.arange(SB)[None, None, :]
    maskt = (128 * t + p <= c).astype(bf)
    ident = np.eye(128, dtype=bf)
    wq = np.asarray(wq, np.float32)
    wk = np.asarray(wk, np.float32)
    wv = np.asarray(wv, np.float32)
    wo = np.asarray(wo, np.float32)
    in_maps = []
    for i in range(NCORES):
        in_maps.append(
            {
                "xT": xT,
                "cos2": cos2.astype(bf),
                "sin2": sin2.astype(bf),
                "maskt": maskt,
                "ident": ident,
                "wq": np.ascontiguousarray(
                    wq[:, i * HQL * HD : (i + 1) * HQL * HD]
                ).astype(bf),
                "wk": np.ascontiguousarray(wk[:, i * HD : (i + 1) * HD]).astype(bf),
                "wv": np.ascontiguousarray(wv[:, i * HD : (i + 1) * HD]).astype(bf),
                "wo": np.ascontiguousarray(
                    wo[:, i * HQL * HD : (i + 1) * HQL * HD]
                ).astype(bf),
            }
        )
    return in_maps


def _install_trace_shims():
    """The container's antenv lacks axon_hooks; replicate trn_boot's ctypes
    NTFF hook so run_bass_kernel_spmd(trace=True) works. Also stub out the
    fish-bucket artifact upload (no bucket access here)."""
    import sys
    import types
    import ctypes
    import contextlib

    if "antenv.axon_hooks" not in sys.modules:
        mod = types.ModuleType("antenv.axon_hooks")
        mod._hook = None

        def set_axon_ntff_profile_hook(h):
            mod._hook = h

        def get_axon_ntff_profile_hook():
            return mod._hook

        mod.set_axon_ntff_profile_hook = set_axon_ntff_profile_hook
        mod.get_axon_ntff_profile_hook = get_axon_ntff_profile_hook
        sys.modules["antenv.axon_hooks"] = mod

        so_path = "/opt/axon/libaxon_pjrt.so"
        lib = ctypes.CDLL(so_path)
        if hasattr(lib, "axon_start_nrt_profile"):
            lib.axon_start_nrt_profile.argtypes = [
                ctypes.POINTER(ctypes.c_int64),
                ctypes.c_size_t,
            ]
            lib.axon_start_nrt_profile.restype = ctypes.c_int64
            lib.axon_stop_nrt_profile.argtypes = [ctypes.c_char_p]
            lib.axon_stop_nrt_profile.restype = ctypes.c_int64

            @contextlib.contextmanager
            def _hook(output_dir, device_ids):
                import jax

                jax.devices()
                if device_ids:
                    ids = (ctypes.c_int64 * len(device_ids))(*device_ids)
                    rc = lib.axon_start_nrt_profile(ids, len(device_ids))
                else:
                    rc = lib.axon_start_nrt_profile(None, 0)
                if rc != 0:
                    raise RuntimeError(f"axon_start_nrt_profile rc={rc}")
                try:
                    yield
                finally:
                    n = lib.axon_stop_nrt_profile(str(output_dir).encode())
                    if n <= 0:
                        print(f"WARNING: axon_stop_nrt_profile rc={n}")

            set_axon_ntff_profile_hook(_hook)

    import concourse.bass_utils as bu

    bu.upload_artifacts = lambda tmpdir: "local://" + str(tmpdir)


def run(inputs, trace=False, **kw):
    nc = _get_nc()
    if trace:
        _install_trace_shims()
    in_maps = _prep_in_maps(**inputs)
    res = run_bass_kernel_spmd(nc, in_maps, list(range(NCORES)), trace=trace, **kw)
    out = np.concatenate(
        [res.results[i]["outT"].T for i in range(NCORES)], axis=1
    )
    return out.reshape(B, S, D).astype(np.float32), res


def kernel(x, freqs_cos, freqs_sin, wq, wk, wv, wo):
    out, _ = run(
        dict(
            x=x,
            freqs_cos=freqs_cos,
            freqs_sin=freqs_sin,
            wq=wq,
            wk=wk,
            wv=wv,
            wo=wo,
        )
    )
    return out



# revision 7
# speedup vs baseline: 1.0351x; 1.0351x over previous
"""GQA attention + RoPE + causal softmax + output projection on 8 TRN2 cores.

Sharding: tensor-parallel over heads. Core i owns q-heads [4i, 4i+4) and
kv-head i (GQA group size 4 aligns exactly with HQ/8=4, HK/8=1).

Per-core pipeline (everything in transposed "feature-on-partitions" layout),
with stages A (projections+RoPE) and B (attention) interleaved per 512-wide
seq block so the per-(head,qblock) AllGathers start early and finish long
before stage D consumes them:

  for sb in 0..3:
    A(sb): Q^T/K^T/V^T projections for seq block sb (lhsT = weight chunk
      [Dc,128], rhs = x^T chunk [Dc,512] -> PSUM [feat,seq]); RoPE on Q^T/K^T
      via stream_shuffle + 2 muls + add; V^T PE-transposed to V [seq,128].
    B(qb=sb): per head, causal attention over k-chunks 0..4*(qb+1):
      scores^T [sk,128 x sq,512] = K^T-chunk (stationary) x Q^T (moving);
      p = exp(scores * 1/sqrt(hd)) on ACT. Diagonal chunks are column-trimmed
      (only sq >= 128*td is computed) into persistent zero-prefix tiles, and
      masked with one [128,128] triangular strip mask (DVE).
      Softmax denominator: exp chunks are pairwise tree-summed on DVE (bf16),
      then ONE all-ones [128,128] stationary matmul on the root broadcasts
      the column sums to every partition (vs one matmul per chunk before:
      removes 160 PE matmuls).
      out^T[128,sq] += V-chunk^T @ p (PE), 2-chunk software pipeline;
      attn^T = out^T * reciprocal_approx_fast(norm) (DVE) -> DMA (vector
      queue) -> per-(h,qb) AllGather [128,512]->[1024,512] on gpsimd.
  D: out^T column shard: lhsT = wo chunk, rhs = gathered attn^T chunk for
     seq block g (depends only on gathers (h, qb=g) - all long done),
     accumulated over all 4096 contraction rows. Host transposes + concats.

PSUM tags are shared across stages (8 banks total): A accumulators
psq0-3/psk/psv, B score tiles rotate over psq0-3, B out/norm use psk/psv and
extra0, V-transpose uses its own bank, D accumulators rotate over psq0-3.

Matmul operands are bf16 (1 cycle/row on PE); accumulation is fp32 in PSUM;
softmax denominator and normalization stay fp32 after the bf16 chunk tree.
"""

import numpy as np
import ml_dtypes

import concourse.bass as bass
import concourse.mybir as mybir
import concourse.tile as tile
from concourse import bacc
from concourse.bass_utils import run_bass_kernel_spmd

# Problem dims (hardcoded per contract)
B, S, D = 1, 2048, 4096
HQ, HK, HD = 32, 8, 128
NCORES = 8
HQL = HQ // NCORES          # 4 local q heads
SB = 512                    # seq block (matmul moving free dim)
NB = S // SB                # 4 seq blocks
NC_ = D // 128              # 32 contraction chunks for D
SCALE = 1.0 / float(np.sqrt(HD))

F32 = mybir.dt.float32
BF16 = mybir.dt.bfloat16

# stream_shuffle mask: swap adjacent pairs within each 32-partition quadrant
SWAP_MASK = [(i ^ 1) for i in range(32)]


def _build_nc():
    nc = bacc.Bacc(
        "TRN2", target_bir_lowering=False, debug=False, num_devices=NCORES
    )

    io = {}
    io["xT"] = nc.dram_tensor("xT", [D, S], BF16, kind="ExternalInput")
    io["wq"] = nc.dram_tensor("wq", [D, HQL * HD], BF16, kind="ExternalInput")
    io["wk"] = nc.dram_tensor("wk", [D, HD], BF16, kind="ExternalInput")
    io["wv"] = nc.dram_tensor("wv", [D, HD], BF16, kind="ExternalInput")
    io["wo"] = nc.dram_tensor("wo", [D, HQL * HD], BF16, kind="ExternalInput")
    io["cos2"] = nc.dram_tensor("cos2", [HD, S], BF16, kind="ExternalInput")
    io["sin2"] = nc.dram_tensor("sin2", [HD, S], BF16, kind="ExternalInput")
    io["maskt"] = nc.dram_tensor("maskt", [128, 128], BF16, kind="ExternalInput")
    io["ident"] = nc.dram_tensor("ident", [128, 128], BF16, kind="ExternalInput")
    io["outT"] = nc.dram_tensor("outT", [HQL * HD, S], F32, kind="ExternalOutput")

    with tile.TileContext(nc) as tc:
        _body(tc, io)
    nc.compile()
    return nc


def _body(tc, io):
    nc = tc.nc
    from contextlib import ExitStack

    ctx = ExitStack()
    with ctx:
        consts = ctx.enter_context(tc.tile_pool(name="consts", bufs=1))
        qkv = ctx.enter_context(tc.tile_pool(name="qkv", bufs=1))
        dram = ctx.enter_context(tc.tile_pool(name="dram", bufs=1, space="DRAM"))
        wpool = ctx.enter_context(tc.tile_pool(name="wpool", bufs=1))
        xpool = ctx.enter_context(tc.tile_pool(name="xpool", bufs=12))
        rpool = ctx.enter_context(tc.tile_pool(name="rpool", bufs=3))
        ppool = ctx.enter_context(tc.tile_pool(name="ppool", bufs=6))
        accp = ctx.enter_context(tc.tile_pool(name="accp", bufs=2))
        spool = ctx.enter_context(tc.tile_pool(name="spool", bufs=2))
        apool = ctx.enter_context(tc.tile_pool(name="apool", bufs=10))
        opool = ctx.enter_context(tc.tile_pool(name="opool", bufs=4))
        ps = ctx.enter_context(tc.tile_pool(name="ps", bufs=1, space="PSUM"))

        # ---- constants ----
        cos2 = consts.tile([HD, S], BF16)
        sin2 = consts.tile([HD, S], BF16)
        ident = consts.tile([128, 128], BF16)
        maskt = consts.tile([128, 128], BF16)
        ones_mat = consts.tile([128, 128], BF16)
        nc.vector.memset(ones_mat, 1.0)

        # ---- persistent per-core tensors ----
        qt_sb = [
            qkv.tile([HD, HQL, SB], BF16, name=f"qt{sb}") for sb in range(NB)
        ]
        kt_sb = [qkv.tile([HD, SB], BF16, name=f"kt{sb}") for sb in range(NB)]
        vs_sb = [
            qkv.tile([128, SB // 128, HD], BF16, name=f"vs{sb}") for sb in range(NB)
        ]
        # persistent zero-prefix exp tiles for the 4 diagonal chunk offsets;
        # prefix [0, 128*td) is zeroed once and never written again, so the
        # trimmed exp + full-width PV/denominator accumulation stay correct.
        ptd = [qkv.tile([128, SB], BF16, name=f"ptd{td}") for td in range(4)]
        for td in range(1, 4):
            nc.vector.memset(ptd[td][:, : 128 * td], 0.0)

        # per-(head, qblock) bounce + gather buffers (fine-grained collectives
        # issued as each (h, qb) finishes -> fully hidden under later compute)
        attn_loc = [
            [dram.tile([HD, SB], BF16, name=f"aloc{h}_{qb}") for qb in range(NB)]
            for h in range(HQL)
        ]
        attn_g = [
            [
                dram.tile(
                    [NCORES * HD, SB], BF16, name=f"ag{h}_{qb}", addr_space="Shared"
                )
                for qb in range(NB)
            ]
            for h in range(HQL)
        ]

        # ---- weight loads: first chunks spread across queues so the PE
        # starts ASAP; bulk on gpsimd (done during A(0), before collectives)
        wq_sb = wpool.tile([128, NC_, HQL * HD], BF16)
        wk_sb = wpool.tile([128, NC_, HD], BF16)
        wv_sb = wpool.tile([128, NC_, HD], BF16)
        for t in range(HQL):
            eng = [nc.scalar, nc.scalar, nc.sync, nc.sync][t]
            eng.dma_start(
                out=wq_sb[:, 0, t * 128 : (t + 1) * 128],
                in_=io["wq"][0:128, t * 128 : (t + 1) * 128],
            )
        nc.gpsimd.dma_start(out=wk_sb[:, 0, :], in_=io["wk"][0:128, :])
        nc.gpsimd.dma_start(out=wv_sb[:, 0, :], in_=io["wv"][0:128, :])
        nc.gpsimd.dma_start(out=maskt, in_=io["maskt"][:, :])
        for c in range(1, 4):
            sl = slice(c * 128, (c + 1) * 128)
            nc.gpsimd.dma_start(out=wq_sb[:, c, :], in_=io["wq"][sl, :])
            nc.gpsimd.dma_start(out=wk_sb[:, c, :], in_=io["wk"][sl, :])
            nc.gpsimd.dma_start(out=wv_sb[:, c, :], in_=io["wv"][sl, :])
        for c4 in range(1, NC_ // 4):
            sl = slice(c4 * 4 * 128, (c4 * 4 + 4) * 128)
            nc.gpsimd.dma_start(
                out=wq_sb[:, c4 * 4 : c4 * 4 + 4, :],
                in_=io["wq"][sl, :].rearrange("(c p) n -> p c n", p=128),
            )
            nc.gpsimd.dma_start(
                out=wk_sb[:, c4 * 4 : c4 * 4 + 4, :],
                in_=io["wk"][sl, :].rearrange("(c p) n -> p c n", p=128),
            )
            nc.gpsimd.dma_start(
                out=wv_sb[:, c4 * 4 : c4 * 4 + 4, :],
                in_=io["wv"][sl, :].rearrange("(c p) n -> p c n", p=128),
            )

        wo_sb = wpool.tile([128, NC_, HQL * HD], BF16)

        # =============== interleaved stage A(sb) + stage B(qb=sb) ===========
        for sb in range(NB):
            _stage_a_block(nc, tc, io, sb, ps, xpool, rpool,
                           (wq_sb, wk_sb, wv_sb),
                           cos2, sin2, ident, qt_sb, kt_sb, vs_sb)
            if sb == 0:
                # wo loads fill DMA idle time from here on (gpsimd queue,
                # after the qkv weights and before most collectives)
                for c4 in range(NC_ // 4):
                    sl = slice(c4 * 4 * 128, (c4 * 4 + 4) * 128)
                    nc.gpsimd.dma_start(
                        out=wo_sb[:, c4 * 4 : c4 * 4 + 4, :],
                        in_=io["wo"][sl, :].rearrange("(c p) n -> p c n", p=128),
                    )
            _stage_b_block(nc, tc, sb, ps, ppool, accp, spool, maskt, ones_mat,
                           qt_sb, kt_sb, vs_sb, ptd, attn_loc, attn_g)

        # ================= Stage D: out = attn @ wo (column shard) =========
        for g in range(NB):
            gsl = slice(g * SB, (g + 1) * SB)
            ps_d = [
                ps.tile([128, SB], F32, name=f"psd{g}_{n}", tag=f"psq{n}")
                for n in range(HQL)
            ]
            for j in range(HQL):
                for i in range(NCORES):
                    c = i * HQL + j
                    at = apool.tile([128, SB], BF16, tag="at")
                    nc.sync.dma_start(
                        out=at, in_=attn_g[j][g][i * 128 : (i + 1) * 128, :]
                    )
                    first = j == 0 and i == 0
                    last = j == HQL - 1 and i == NCORES - 1
                    for n in range(HQL):
                        nc.tensor.matmul(
                            ps_d[n],
                            lhsT=wo_sb[:, c, n * 128 : (n + 1) * 128],
                            rhs=at,
                            start=first,
                            stop=last,
                        )
            for n in range(HQL):
                ot = opool.tile([128, SB], F32, name=f"ot{g}_{n}", tag="ot")
                nc.scalar.copy(ot, ps_d[n])
                nc.sync.dma_start(
                    out=io["outT"][n * 128 : (n + 1) * 128, gsl], in_=ot
                )


def _stage_a_block(nc, tc, io, sb, ps, xpool, rpool, weights,
                   cos2, sin2, ident, qt_sb, kt_sb, vs_sb):
    """Projections + RoPE for seq block sb."""
    ssl = slice(sb * SB, (sb + 1) * SB)
    ps_q = [
        ps.tile([128, SB], F32, name=f"psq{t}_{sb}", tag=f"psq{t}")
        for t in range(HQL)
    ]
    ps_k = ps.tile([128, SB], F32, name=f"psk_{sb}", tag="psk")
    ps_v = ps.tile([128, SB], F32, name=f"psv_{sb}", tag="psv")
    wq_sb, wk_sb, wv_sb = weights
    for c in range(NC_):
        xt = xpool.tile([128, SB], BF16, tag="xt")
        xt_eng = nc.sync if c % 2 == 0 else nc.scalar
        xt_eng.dma_start(out=xt, in_=io["xT"][c * 128 : (c + 1) * 128, ssl])
        first, last = c == 0, c == NC_ - 1
        for t in range(HQL):
            nc.tensor.matmul(
                ps_q[t],
                lhsT=wq_sb[:, c, t * 128 : (t + 1) * 128],
                rhs=xt,
                start=first,
                stop=last,
            )
        nc.tensor.matmul(
            ps_k, lhsT=wk_sb[:, c, :], rhs=xt, start=first, stop=last
        )
        nc.tensor.matmul(
            ps_v, lhsT=wv_sb[:, c, :], rhs=xt, start=first, stop=last
        )

    # const loads trail this block's xt DMAs on the scalar queue
    if sb == 0:
        nc.scalar.dma_start(out=ident, in_=io["ident"][:, :])
    nc.scalar.dma_start(out=cos2[:, ssl], in_=io["cos2"][:, ssl])
    nc.scalar.dma_start(out=sin2[:, ssl], in_=io["sin2"][:, ssl])

    # PSUM evictions (ACT copies, bf16 cast). k first for sb==0 so B(0)'s
    # first scores aren't gated on the whole rope chain; q0 first otherwise.
    qc_k = rpool.tile([128, SB], BF16, name=f"qck{sb}", tag="qck")
    qc_q = [
        rpool.tile([128, SB], BF16, name=f"qc{sb}_{t}", tag=f"qc{t}")
        for t in range(HQL)
    ]

    def rope_dve(qc, dst, idx):
        sw = rpool.tile([128, SB], BF16, name=f"sw{idx}", tag="sw")
        nc.vector.stream_shuffle(sw, qc, SWAP_MASK)
        t1 = rpool.tile([128, SB], BF16, name=f"t1{idx}", tag="t1")
        nc.vector.tensor_mul(t1, qc, cos2[:, ssl])
        t2 = rpool.tile([128, SB], BF16, name=f"t2{idx}", tag="t2")
        nc.vector.tensor_mul(t2, sw, sin2[:, ssl])
        nc.vector.tensor_add(dst, t1, t2)

    if sb == 0:
        nc.scalar.copy(qc_k, ps_k)
        rope_dve(qc_k, kt_sb[sb], f"k{sb}")
        nc.scalar.copy(qc_q[0], ps_q[0])
        rope_dve(qc_q[0], qt_sb[sb][:, 0, :], f"q{sb}_0")
    else:
        nc.scalar.copy(qc_q[0], ps_q[0])
        rope_dve(qc_q[0], qt_sb[sb][:, 0, :], f"q{sb}_0")
        nc.scalar.copy(qc_k, ps_k)
        rope_dve(qc_k, kt_sb[sb], f"k{sb}")
    for t in range(1, HQL):
        nc.scalar.copy(qc_q[t], ps_q[t])
        rope_dve(qc_q[t], qt_sb[sb][:, t, :], f"q{sb}_{t}")

    # V^T -> V (PE transpose per 128-col chunk)
    vts = rpool.tile([128, SB], BF16, name=f"vts{sb}", tag="vts")
    nc.scalar.copy(vts, ps_v)
    for u in range(SB // 128):
        ps_vt = ps.tile([128, 128], BF16, name=f"psvt{sb}_{u}", tag="psvt")
        nc.tensor.transpose(ps_vt, vts[:, u * 128 : (u + 1) * 128], ident)
        nc.vector.tensor_copy(vs_sb[sb][:, u, :], ps_vt)


def _stage_b_block(nc, tc, qb, ps, ppool, accp, spool, maskt, ones_mat,
                   qt_sb, kt_sb, vs_sb, ptd, attn_loc, attn_g):
    """Causal attention for q-block qb over k-chunks 0..4*(qb+1)."""
    nkc = 4 * (qb + 1)
    for h in range(HQL):
        pso = ps.tile(
            [128, SB], F32, name=f"pso{qb}_{h}", tag=("psk" if h % 2 == 0 else "psv")
        )
        psn = ps.tile([128, SB], F32, name=f"psn{qb}_{h}", tag="extra0")
        pts = {}
        pend = {}  # binary-counter pairwise tree accumulation on DVE

        def feed(t, _s=[0]):
            lvl = 0
            while lvl in pend:
                prev = pend.pop(lvl)
                _s[0] += 1
                nt = accp.tile(
                    [128, SB], BF16, name=f"acc{qb}_{h}_{_s[0]}", tag=f"acc{lvl}"
                )
                nc.vector.tensor_add(nt, prev, t)
                t = nt
                lvl += 1
            pend[lvl] = t

        def pv(kc):
            nc.tensor.matmul(
                pso,
                lhsT=vs_sb[kc // 4][:, kc % 4, :],
                rhs=pts.pop(kc),
                start=kc == 0,
                stop=kc == nkc - 1,
            )

        for kc in range(nkc):
            td = kc - 4 * qb
            pss = ps.tile(
                [128, SB], F32, name=f"pss{qb}_{h}_{kc}", tag=f"psq{kc % 4}"
            )
            ktc = kt_sb[kc // 4][:, (kc % 4) * 128 : (kc % 4 + 1) * 128]
            qtc = qt_sb[qb][:, h, :]
            if td < 0:
                nc.tensor.matmul(pss, lhsT=ktc, rhs=qtc, start=True, stop=True)
                pt = ppool.tile([128, SB], BF16, name=f"pt{qb}_{h}_{kc}", tag="pt")
                nc.scalar.activation(
                    pt, pss, mybir.ActivationFunctionType.Exp, scale=SCALE
                )
            else:
                lo = 128 * td
                nc.tensor.matmul(
                    pss[:, lo:], lhsT=ktc, rhs=qtc[:, lo:], start=True, stop=True
                )
                pt = ptd[td]
                nc.scalar.activation(
                    pt[:, lo:],
                    pss[:, lo:],
                    mybir.ActivationFunctionType.Exp,
                    scale=SCALE,
                )
                # triangular strip mask (in-place on the 128-wide strip)
                nc.vector.tensor_mul(
                    pt[:, lo : lo + 128], pt[:, lo : lo + 128], maskt
                )
            pts[kc] = pt
            feed(pt)
            if kc >= 2:
                pv(kc - 2)
        pv(nkc - 2)
        pv(nkc - 1)
        # combine leftover tree levels ascending -> root
        lvls = sorted(pend)
        root = pend[lvls[0]]
        for lv in lvls[1:]:
            nt = accp.tile([128, SB], BF16, name=f"accr{qb}_{h}_{lv}", tag="accr")
            nc.vector.tensor_add(nt, pend[lv], root)
            root = nt
        # ones_mat stationary => every partition of psn gets the column-sum:
        # the softmax denominator, already broadcast.
        nc.tensor.matmul(psn, lhsT=ones_mat, rhs=root, start=True, stop=True)
        rb = spool.tile([128, SB], F32, name=f"rb{qb}_{h}", tag="rb")
        nc.vector.reciprocal_approx_fast(rb, psn)
        ao = spool.tile([128, SB], BF16, name=f"ao{qb}_{h}", tag="ao", bufs=4)
        nc.vector.tensor_mul(ao, pso, rb)
        # ao DMA on gpsimd: precedes its gather trigger on the same queue
        nc.gpsimd.dma_start(out=attn_loc[h][qb][:, :], in_=ao)
        nc.gpsimd.collective_compute(
            "AllGather",
            mybir.AluOpType.bypass,
            replica_groups=[list(range(NCORES))],
            ins=[attn_loc[h][qb].opt()],
            outs=[attn_g[h][qb].opt()],
        )


_NC_CACHE = None


def _get_nc():
    global _NC_CACHE
    if _NC_CACHE is None:
        _NC_CACHE = _build_nc()
    return _NC_CACHE


def _prep_in_maps(x, freqs_cos, freqs_sin, wq, wk, wv, wo):
    bf = ml_dtypes.bfloat16
    x = np.asarray(x, np.float32).reshape(S, D)
    xT = np.ascontiguousarray(x.T).astype(bf)
    cos = np.asarray(freqs_cos, np.float32)  # [S, HD/2]
    sin = np.asarray(freqs_sin, np.float32)
    cos2 = np.repeat(cos.T, 2, axis=0)  # [HD, S], rows 2j,2j+1 = cos[:, j]
    sin_t = sin.T
    sin2 = np.empty((HD, S), np.float32)
    sin2[0::2] = -sin_t
    sin2[1::2] = sin_t
    p = np.arange(128)[:, None]
    c = np.arange(128)[None, :]
    maskt = (p <= c).astype(bf)  # triangular strip mask, same for every td
    ident = np.eye(128, dtype=bf)
    wq = np.asarray(wq, np.float32)
    wk = np.asarray(wk, np.float32)
    wv = np.asarray(wv, np.float32)
    wo = np.asarray(wo, np.float32)
    in_maps = []
    for i in range(NCORES):
        in_maps.append(
            {
                "xT": xT,
                "cos2": cos2.astype(bf),
                "sin2": sin2.astype(bf),
                "maskt": maskt,
                "ident": ident,
                "wq": np.ascontiguousarray(
                    wq[:, i * HQL * HD : (i + 1) * HQL * HD]
                ).astype(bf),
                "wk": np.ascontiguousarray(wk[:, i * HD : (i + 1) * HD]).astype(bf),
                "wv": np.ascontiguousarray(wv[:, i * HD : (i + 1) * HD]).astype(bf),
                "wo": np.ascontiguousarray(
                    wo[:, i * HQL * HD : (i + 1) * HQL * HD]
                ).astype(bf),
            }
        )
    return in_maps


def _install_trace_shims():
    """The container's antenv lacks axon_hooks; replicate trn_boot's ctypes
    NTFF hook so run_bass_kernel_spmd(trace=True) works. Also stub out the
    fish-bucket artifact upload (no bucket access here)."""
    import sys
    import types
    import ctypes
    import contextlib

    if "antenv.axon_hooks" not in sys.modules:
        mod = types.ModuleType("antenv.axon_hooks")
        mod._hook = None

        def set_axon_ntff_profile_hook(h):
            mod._hook = h

        def get_axon_ntff_profile_hook():
            return mod._hook

        mod.set_axon_ntff_profile_hook = set_axon_ntff_profile_hook
        mod.get_axon_ntff_profile_hook = get_axon_ntff_profile_hook
        sys.modules["antenv.axon_hooks"] = mod

        so_path = "/opt/axon/libaxon_pjrt.so"
        lib = ctypes.CDLL(so_path)
        if hasattr(lib, "axon_start_nrt_profile"):
            lib.axon_start_nrt_profile.argtypes = [
                ctypes.POINTER(ctypes.c_int64),
                ctypes.c_size_t,
            ]
            lib.axon_start_nrt_profile.restype = ctypes.c_int64
            lib.axon_stop_nrt_profile.argtypes = [ctypes.c_char_p]
            lib.axon_stop_nrt_profile.restype = ctypes.c_int64

            @contextlib.contextmanager
            def _hook(output_dir, device_ids):
                import jax

                jax.devices()
                if device_ids:
                    ids = (ctypes.c_int64 * len(device_ids))(*device_ids)
                    rc = lib.axon_start_nrt_profile(ids, len(device_ids))
                else:
                    rc = lib.axon_start_nrt_profile(None, 0)
                if rc != 0:
                    raise RuntimeError(f"axon_start_nrt_profile rc={rc}")
                try:
                    yield
                finally:
                    n = lib.axon_stop_nrt_profile(str(output_dir).encode())
                    if n <= 0:
                        print(f"WARNING: axon_stop_nrt_profile rc={n}")

            set_axon_ntff_profile_hook(_hook)

    import concourse.bass_utils as bu

    bu.upload_artifacts = lambda tmpdir: "local://" + str(tmpdir)


def run(inputs, trace=False, **kw):
    nc = _get_nc()
    if trace:
        _install_trace_shims()
    in_maps = _prep_in_maps(**inputs)
    res = run_bass_kernel_spmd(nc, in_maps, list(range(NCORES)), trace=trace, **kw)
    out = np.concatenate(
        [res.results[i]["outT"].T for i in range(NCORES)], axis=1
    )
    return out.reshape(B, S, D).astype(np.float32), res


def kernel(x, freqs_cos, freqs_sin, wq, wk, wv, wo):
    out, _ = run(
        dict(
            x=x,
            freqs_cos=freqs_cos,
            freqs_sin=freqs_sin,
            wq=wq,
            wk=wk,
            wv=wv,
            wo=wo,
        )
    )
    return out
